# revision 9
# baseline (speedup 1.0000x reference)
"""MultiHeadLatentAttention TRN2 kernel.

Sharding: 8 cores = 2 batches x 4 head-groups (4 heads of 128 dims each).
Each core computes, for its (batch, 4 heads):
    qT_h = Wq_h^T xT          [hd, S]     (fp16 matmuls, fp32 psum)
    latT = Wdkv^T xT          [256, S]
    kT_h = Wuk_h^T latT       [hd, S]
    v    = latT^T Wuv         [S, 4*hd]   (per S-tile, all 4 heads wide)
    scoresT = k qT            [keys, q]   (transposed scores: no transposes)
    exT = exp(scale*scoresT)  per key-tile on exact valid column ranges
    exacc += exT              (DVE accumulate over key tiles)
    den  = ones^T exacc       [128, q]    (one matmul per (h,qb): sum over
                                           keys AND broadcast across parts)
    ctxT = v^T exT / den      [hd, q]
    part = sum_h ctxT_h^T Wout_h  [S, dout]  (row-parallel partial, fp16)
Host sums the 4 partials per batch and adds b_out.
"""

import sys

_BASS_REPO = "/opt/trn_rl_repo"
if _BASS_REPO not in sys.path:
    sys.path.insert(0, _BASS_REPO)

import numpy as np

import concourse.bass as bass  # noqa: F401
import concourse.mybir as mybir
import concourse.tile as tile
from concourse import bacc, bass_utils

F32 = mybir.dt.float32
F16 = mybir.dt.float16

B = 2
S = 2048
DIN = 2048
DOUT = 2048
NH = 16
HD = 128
LAT = 256
NCORES = 8
HEADS_PER_CORE = 4
COLS_PER_CORE = HEADS_PER_CORE * HD  # 512

KC = DIN // 128  # 16 contraction chunks over d_in
NB = S // 512    # 4 blocks of 512 over S
NT = S // 128    # 16 tiles of 128 over S
SCALE = 1.0 / float(np.sqrt(HD))

_CACHE = {}


def _build():
    nc = bacc.Bacc("TRN2", target_bir_lowering=False, debug=False,
                   num_devices=NCORES)

    xt_d = nc.dram_tensor("xt", [DIN, S], F16, kind="ExternalInput")
    wq_d = nc.dram_tensor("wq", [DIN, COLS_PER_CORE], F16, kind="ExternalInput")
    wdkv_d = nc.dram_tensor("wdkv", [DIN, LAT], F16, kind="ExternalInput")
    wuk_d = nc.dram_tensor("wuk", [LAT, COLS_PER_CORE], F16, kind="ExternalInput")
    wuv_d = nc.dram_tensor("wuv", [LAT, COLS_PER_CORE], F16, kind="ExternalInput")
    wout_d = nc.dram_tensor("wout", [COLS_PER_CORE, DOUT], F16, kind="ExternalInput")
    mask_d = nc.dram_tensor("mask", [128, 128], F16, kind="ExternalInput")
    out_d = nc.dram_tensor("out", [S, DOUT], F16, kind="ExternalOutput")

    Exp = mybir.ActivationFunctionType.Exp

    with tile.TileContext(nc) as tc:
        with (
            tc.tile_pool(name="consts", bufs=1) as cpool,
            tc.tile_pool(name="wts", bufs=1) as wpool,
            tc.tile_pool(name="acts", bufs=1) as apool,
            tc.tile_pool(name="temps", bufs=1) as tpool,
        ):
            # ---- constants ----
            ones_t = cpool.tile([128, 512], F16, name="ones_t", tag="ones_t")
            nc.vector.memset(ones_t[:], 1.0)
            mask_t = cpool.tile([128, 128], F16, name="mask_t", tag="mask_t")
            nc.scalar.dma_start(mask_t[:], mask_d.ap())

            # ---- weights ----
            # xt/wdkv stream on the sync HWDGE ring (feed the first matmuls);
            # everything else loads in parallel on the scalar ring.
            # xt chunk 0 is split across both rings to cut first-matmul
            # latency.
            wdkv = []
            xt = []
            # Phase-1 critical stream: ONLY xt+wdkv ride the queues while the
            # latT pass runs.  Per-queue effective rate is ~150-160 GB/s
            # (in-flight credit x completion latency), so the ~316 GB/s
            # latT-pass demand is split over THREE queues (sync/scalar HWDGE
            # + gpsimd SWDGE): 224/176/176 KB per 1.79us chunk window.
            # Chunk 0 split finer so the first matmul's inputs land ~1.7us.
            for k in range(KC):
                td = wpool.tile([128, LAT], F16, name=f"wdkv{k}", tag=f"wdkv{k}")
                nc.sync.dma_start(td[:], wdkv_d.ap()[128 * k:128 * (k + 1), :])
                wdkv.append(td)
                t = wpool.tile([128, S], F16, name=f"xt{k}", tag=f"xt{k}")
                r = xt_d.ap()[128 * k:128 * (k + 1), :]
                if k == 0:
                    nc.sync.dma_start(t[:, 0:512], r[:, 0:512])
                    nc.sync.dma_start(t[:, 512:640], r[:, 512:640])
                    nc.scalar.dma_start(t[:, 640:1152], r[:, 640:1152])
                    nc.scalar.dma_start(t[:, 1152:1344], r[:, 1152:1344])
                    nc.gpsimd.dma_start(t[:, 1344:2048], r[:, 1344:2048])
                else:
                    nc.sync.dma_start(t[:, 0:640], r[:, 0:640])
                    nc.scalar.dma_start(t[:, 640:1344], r[:, 640:1344])
                    nc.gpsimd.dma_start(t[:, 1344:2048], r[:, 1344:2048])
                xt.append(t)
            # wuk/wuv queue on sync AFTER its xt share (arrive ~32us, needed
            # ~86us). wq/wout are deferred until after the latT pass is
            # emitted (see below) so they stream on scalar behind xt.
            wuk = []
            wuv = []
            for m in range(LAT // 128):
                t = wpool.tile([128, COLS_PER_CORE], F16, name=f"wuk{m}", tag=f"wuk{m}")
                nc.sync.dma_start(t[:], wuk_d.ap()[128 * m:128 * (m + 1), :])
                wuk.append(t)
                t = wpool.tile([128, COLS_PER_CORE], F16, name=f"wuv{m}", tag=f"wuv{m}")
                nc.sync.dma_start(t[:], wuv_d.ap()[128 * m:128 * (m + 1), :])
                wuv.append(t)
            wq = [wpool.tile([128, COLS_PER_CORE], F16, name=f"wq{k}", tag=f"wq{k}")
                  for k in range(KC)]
            wout = [wpool.tile([128, DOUT], F16, name=f"wout{h}", tag=f"wout{h}")
                    for h in range(HEADS_PER_CORE)]

            # ---- persistent activations ----
            latT = [apool.tile([128, S], F16, name=f"latT{m}", tag=f"latT{m}")
                    for m in range(LAT // 128)]
            qT = [apool.tile([128, S], F16, name=f"qT{h}", tag=f"qT{h}")
                  for h in range(HEADS_PER_CORE)]
            kT = [apool.tile([128, S], F16, name=f"kT{h}", tag=f"kT{h}")
                  for h in range(HEADS_PER_CORE)]
            # vtt[stt]: [s within tile (part), 4 heads x hd (free)]
            vtt = [apool.tile([128, 512], F16, name=f"vtt{t}", tag=f"vtt{t}")
                   for t in range(NT)]
            ctxT = [apool.tile([128, S], F16, name=f"ctxT{h}", tag=f"ctxT{h}")
                    for h in range(HEADS_PER_CORE)]

            # ================= phase 1: projections =================
            with tc.tile_pool(name="pproj", bufs=8, space="PSUM") as pproj:
                # PE warmup: HAM-warm the array while input DMAs stream in.
                # ~32 cold matmuls (~3.4us at 1.2GHz) bridges to the first
                # xt piece landing AND spans the HAM activity window.
                warm = pproj.tile([128, 512], F32, name="warm", tag="pp")
                for _ in range(32):
                    nc.tensor.matmul(warm[:, 0:128], ones_t[:, 0:128],
                                     ones_t[:, 0:128], start=True, stop=True)

                def kmajor(groups, lhs_of, rhs_of, nk, out_of, copy_eng="alt"):
                    """Accumulate len(groups) psum banks over nk chunks,
                    chunk-major so compute starts on the first DMA. The
                    drain copies alternate scalar/vector so the next pass's
                    matmuls get psum banks back at 2x the single-engine
                    copy rate."""
                    pls = [pproj.tile([128, 512], F32, name=f"pp{i}", tag="pp")
                           for i in range(len(groups))]
                    for k in range(nk):
                        for i, g in enumerate(groups):
                            nc.tensor.matmul(pls[i][:], lhs_of(k, g), rhs_of(k, g),
                                             start=(k == 0), stop=(k == nk - 1))
                    for i, g in enumerate(groups):
                        if i % 2 == 0:
                            nc.scalar.copy(out_of(g), pls[i][:])
                        else:
                            nc.vector.tensor_copy(out_of(g), pls[i][:])

                # latT = Wdkv^T xT   (8 groups: 2 m x 4 sb)
                kmajor(
                    [(m, sb) for m in range(2) for sb in range(NB)],
                    lambda k, g: wdkv[k][:, 128 * g[0]:128 * (g[0] + 1)],
                    lambda k, g: xt[k][:, 512 * g[1]:512 * (g[1] + 1)],
                    KC,
                    lambda g: latT[g[0]][:, 512 * g[1]:512 * (g[1] + 1)])

                # wq/wout stream on scalar BEHIND its xt share (FIFO per
                # ring): wq chunk k lands ~23+0.7k us, needed ~29+1.8k us.
                for k in range(KC):
                    nc.scalar.dma_start(wq[k][:],
                                        wq_d.ap()[128 * k:128 * (k + 1), :])
                for h in range(HEADS_PER_CORE):
                    nc.scalar.dma_start(wout[h][:],
                                        wout_d.ap()[128 * h:128 * (h + 1), :])

                # qT_h = Wq_h^T xT   (two batches of 8 groups: 2 h x 4 sb)
                for h0 in (0, 2):
                    kmajor(
                        [(h0 + dh, sb) for dh in range(2) for sb in range(NB)],
                        lambda k, g: wq[k][:, 128 * g[0]:128 * (g[0] + 1)],
                        lambda k, g: xt[k][:, 512 * g[1]:512 * (g[1] + 1)],
                        KC,
                        lambda g: qT[g[0]][:, 512 * g[1]:512 * (g[1] + 1)])

                # kT_h = Wuk_h^T latT
                kmajor(
                    [(h, sb) for h in range(2) for sb in range(NB)],
                    lambda k, g: wuk[k][:, 128 * g[0]:128 * (g[0] + 1)],
                    lambda k, g: latT[k][:, 512 * g[1]:512 * (g[1] + 1)],
                    2,
                    lambda g: kT[g[0]][:, 512 * g[1]:512 * (g[1] + 1)])
                kmajor(
                    [(h, sb) for h in (2, 3) for sb in range(NB)],
                    lambda k, g: wuk[k][:, 128 * g[0]:128 * (g[0] + 1)],
                    lambda k, g: latT[k][:, 512 * g[1]:512 * (g[1] + 1)],
                    2,
                    lambda g: kT[g[0]][:, 512 * g[1]:512 * (g[1] + 1)])

                # v = latT^T Wuv per S-tile, all 4 heads wide (N=512).
                # Only tiles 0-3 here (qb0 reads them); tiles 4-15 are
                # emitted as interleaved filler inside qb0's attention.
                for stt in range(4):
                    pv = pproj.tile([128, 512], F32, name="pv", tag="pp")
                    for m in range(LAT // 128):
                        nc.tensor.matmul(
                            pv[:],
                            latT[m][:, 128 * stt:128 * (stt + 1)],
                            wuv[m][:],
                            start=(m == 0), stop=(m == LAT // 128 - 1))
                    if stt % 2 == 0:
                        nc.vector.tensor_copy(vtt[stt][:], pv[:])
                    else:
                        nc.scalar.copy(vtt[stt][:], pv[:])

            # ========= phase 2: attention + interleaved out-proj =========
            # per key-tile processing on exact valid column ranges; softmax
            # denominator accumulated on DVE (exacc), one ones-matmul per
            # (h, qb) to reduce over keys + broadcast across partitions.
            # qb-outer / h-inner so each q-block's out-projection (PE-heavy,
            # ACT-idle) overlaps the next block's ACT-paced attention.
            # Full-row output staging: 4 psum drains land in one [128, 2048]
            # tile, then ONE 512KB fully-contiguous DMA per stt (rotating
            # across the three DMA queues) instead of 4x128KB strided.
            osb_cur = {}
            out_qs = [nc.sync, nc.scalar, nc.gpsimd]

            with (
                tc.tile_pool(name="psc", bufs=4, space="PSUM") as psc,
                tc.tile_pool(name="pctx", bufs=2, space="PSUM") as pctx,
                tc.tile_pool(name="pden", bufs=2, space="PSUM") as pden,
            ):
                def emit_outproj(stt, ob, pool, tail):
                    po = pool.tile([128, 512], F32, name="po", tag="den")
                    for h in range(HEADS_PER_CORE):
                        nc.tensor.matmul(
                            po[:],
                            ctxT[h][:, 128 * stt:128 * (stt + 1)],
                            wout[h][:, 512 * ob:512 * (ob + 1)],
                            start=(h == 0), stop=(h == HEADS_PER_CORE - 1))
                    if ob == 0:
                        osb_cur[stt] = tpool.tile([128, DOUT], F16, name="osb",
                                                  tag="osb", bufs=3)
                    osb = osb_cur[stt]
                    if tail and ob % 2 == 0:
                        nc.scalar.copy(osb[:, 512 * ob:512 * (ob + 1)], po[:])
                    else:
                        nc.vector.tensor_copy(osb[:, 512 * ob:512 * (ob + 1)],
                                              po[:])
                    if ob == NB - 1:
                        out_qs[stt % 3].dma_start(
                            out_d.ap()[128 * stt:128 * (stt + 1), :], osb[:])

                # out-projection of block qb-1 is interleaved into block
                # qb's attention (one (stt, ob) group every few key-tiles)
                # so the PE fills exp-latency bubbles with out-proj matmuls
                # and (h, qb) boundaries never drain the pipeline.
                def emit_vop(stt):
                    pv = psc.tile([128, 512], F32, name="pv", tag="sc")
                    for m in range(LAT // 128):
                        nc.tensor.matmul(
                            pv[:],
                            latT[m][:, 128 * stt:128 * (stt + 1)],
                            wuv[m][:],
                            start=(m == 0), stop=(m == LAT // 128 - 1))
                    if stt % 2 == 0:
                        nc.vector.tensor_copy(vtt[stt][:], pv[:])
                    else:
                        nc.scalar.copy(vtt[stt][:], pv[:])

                for qb in range(NB):
                    if qb > 0:
                        ops = [(lambda s=stt, o=ob:
                                emit_outproj(s, o, pden, tail=False))
                               for stt in range(4 * (qb - 1), 4 * qb)
                               for ob in range(NB)]
                    else:
                        # fill qb0's exp-latency bubbles with the remaining
                        # v-tile projections (first needed by qb1)
                        ops = [(lambda s=stt: emit_vop(s))
                               for stt in range(4, NT)]
                    nkt = 4 * qb + 4
                    total_kts = HEADS_PER_CORE * nkt
                    emitted = 0
                    done_kts = 0
                    for h in range(HEADS_PER_CORE):
                        ps_ctx = pctx.tile([128, 512], F32, name="ps_ctx", tag="ctx")
                        exacc = tpool.tile([128, 512], F16, name="exacc",
                                           tag="exacc", bufs=3)
                        for kt in range(nkt):
                            dj = kt - 4 * qb
                            c = 128 * dj if dj > 0 else 0
                            ps_sc = psc.tile([128, 512], F32, name="ps_sc",
                                             tag="sc")
                            ex = tpool.tile([128, 512], F16, name="ex", tag="ex",
                                            bufs=6)
                            nc.tensor.matmul(
                                ps_sc[:, c:512],
                                kT[h][:, 128 * kt:128 * (kt + 1)],
                                qT[h][:, 512 * qb + c:512 * (qb + 1)],
                                start=True, stop=True)
                            nc.scalar.activation(ex[:, c:512], ps_sc[:, c:512],
                                                 Exp, scale=SCALE)
                            if dj >= 0:
                                nc.vector.tensor_mul(
                                    ex[:, c:c + 128], ex[:, c:c + 128], mask_t[:])
                            if kt == 0:
                                nc.vector.tensor_copy(exacc[:], ex[:])
                            else:
                                nc.vector.tensor_add(exacc[:, c:512],
                                                     exacc[:, c:512],
                                                     ex[:, c:512])
                            nc.tensor.matmul(
                                ps_ctx[:, c:512],
                                vtt[kt][:, 128 * h:128 * (h + 1)],
                                ex[:, c:512],
                                start=(kt == 0), stop=(kt == nkt - 1),
                                skip_group_check=True)
                            done_kts += 1
                            while ops and emitted < (done_kts * len(ops)) // total_kts:
                                ops[emitted]()
                                emitted += 1
                        ps_den = pden.tile([128, 512], F32, name="ps_den",
                                           tag="den")
                        nc.tensor.matmul(ps_den[:], ones_t[:, 0:128], exacc[:],
                                         start=True, stop=True)
                        rden = tpool.tile([128, 512], F32, name="rden", tag="rden",
                                          bufs=2)
                        nc.vector.reciprocal_approx_fast(rden[:], ps_den[:])
                        nc.vector.tensor_mul(ctxT[h][:, 512 * qb:512 * (qb + 1)],
                                             ps_ctx[:], rden[:])
                    while ops and emitted < len(ops):
                        ops[emitted]()
                        emitted += 1

            # final q-block's out-projection (nothing left to overlap).
            # Runs in its own wide PSUM pool — the attention pools are closed,
            # so 6 of the 8 banks can pipeline matmuls ahead of drains.
            with tc.tile_pool(name="ptail", bufs=6, space="PSUM") as ptail:
                for stt in range(4 * (NB - 1), 4 * NB):
                    for ob in range(NB):
                        emit_outproj(stt, ob, ptail, tail=True)

    nc.compile()
    return nc


def _get_nc():
    if "nc" not in _CACHE:
        _CACHE["nc"] = _build()
    return _CACHE["nc"]


def _make_in_maps(x, W_query, W_DKV, W_UK, W_UV, W_out):
    mask = np.triu(np.ones((128, 128), dtype=np.float16))
    wdkv16 = W_DKV.astype(np.float16)
    xT16 = [x[b].T.astype(np.float16) for b in range(B)]
    in_maps = []
    for c in range(NCORES):
        b = c // 4
        g = c % 4
        cols = slice(512 * g, 512 * (g + 1))
        in_maps.append({
            "xt": xT16[b],
            "wq": W_query[:, cols].astype(np.float16),
            "wdkv": wdkv16,
            "wuk": W_UK[:, cols].astype(np.float16),
            "wuv": W_UV[:, cols].astype(np.float16),
            "wout": W_out[cols, :].astype(np.float16),
            "mask": mask,
        })
    return in_maps


def run_on_device(x, W_query, W_DKV, W_UK, W_UV, W_out, **run_kwargs):
    nc = _get_nc()
    in_maps = _make_in_maps(x, W_query, W_DKV, W_UK, W_UV, W_out)
    return bass_utils.run_bass_kernel_spmd(
        nc, in_maps, core_ids=list(range(NCORES)), **run_kwargs)


def kernel(x, W_query, W_DKV, W_UK, W_UV, W_out, b_out):
    x = np.asarray(x, dtype=np.float32)
    W_query = np.asarray(W_query, dtype=np.float32)
    W_DKV = np.asarray(W_DKV, dtype=np.float32)
    W_UK = np.asarray(W_UK, dtype=np.float32)
    W_UV = np.asarray(W_UV, dtype=np.float32)
    W_out = np.asarray(W_out, dtype=np.float32)
    b_out = np.asarray(b_out, dtype=np.float32)

    res = None
    for attempt in range(3):
        try:
            res = run_on_device(x, W_query, W_DKV, W_UK, W_UV, W_out)
            break
        except Exception:
            if attempt == 2:
                raise
    out = np.empty((B, S, DOUT), dtype=np.float32)
    for b in range(B):
        acc = res.results[4 * b]["out"].astype(np.float32)
        for g in range(1, 4):
            acc += res.results[4 * b + g]["out"].astype(np.float32)
        out[b] = acc + b_out[None, :]
    return out



# revision 13
# speedup vs baseline: 1.0161x; 1.0161x over previous
"""MultiHeadLatentAttention TRN2 kernel.

Sharding: 8 cores = 2 batches x 4 head-groups (4 heads of 128 dims each).
Each core computes, for its (batch, 4 heads):
    qT_h = Wq_h^T xT          [hd, S]     (fp16 matmuls, fp32 psum)
    latT = Wdkv^T xT          [256, S]
    kT_h = Wuk_h^T latT       [hd, S]
    v    = latT^T Wuv         [S, 4*hd]   (per S-tile, all 4 heads wide)
    scoresT = k qT            [keys, q]   (transposed scores: no transposes)
    exT = exp(scale*scoresT)  per key-tile on exact valid column ranges
    exacc += exT              (DVE accumulate over key tiles)
    den  = ones^T exacc       [128, q]    (one matmul per (h,qb): sum over
                                           keys AND broadcast across parts)
    ctxT = v^T exT / den      [hd, q]
    part = sum_h ctxT_h^T Wout_h  [S, dout]  (row-parallel partial, fp16)
Host sums the 4 partials per batch and adds b_out.
"""

import sys

_BASS_REPO = "/opt/trn_rl_repo"
if _BASS_REPO not in sys.path:
    sys.path.insert(0, _BASS_REPO)

import numpy as np

import concourse.bass as bass  # noqa: F401
import concourse.mybir as mybir
import concourse.tile as tile
from concourse import bacc, bass_utils

F32 = mybir.dt.float32
F16 = mybir.dt.float16

B = 2
S = 2048
DIN = 2048
DOUT = 2048
NH = 16
HD = 128
LAT = 256
NCORES = 8
HEADS_PER_CORE = 4
COLS_PER_CORE = HEADS_PER_CORE * HD  # 512

KC = DIN // 128  # 16 contraction chunks over d_in
NB = S // 512    # 4 blocks of 512 over S
NT = S // 128    # 16 tiles of 128 over S
SCALE = 1.0 / float(np.sqrt(HD))

_CACHE = {}


def _build():
    nc = bacc.Bacc("TRN2", target_bir_lowering=False, debug=False,
                   num_devices=NCORES)

    xt_d = nc.dram_tensor("xt", [DIN, S], F16, kind="ExternalInput")
    wq_d = nc.dram_tensor("wq", [DIN, COLS_PER_CORE], F16, kind="ExternalInput")
    wdkv_d = nc.dram_tensor("wdkv", [DIN, LAT], F16, kind="ExternalInput")
    wuk_d = nc.dram_tensor("wuk", [LAT, COLS_PER_CORE], F16, kind="ExternalInput")
    wuv_d = nc.dram_tensor("wuv", [LAT, COLS_PER_CORE], F16, kind="ExternalInput")
    wout_d = nc.dram_tensor("wout", [COLS_PER_CORE, DOUT], F16, kind="ExternalInput")
    mask_d = nc.dram_tensor("mask", [128, 128], F16, kind="ExternalInput")
    out_d = nc.dram_tensor("out", [S, DOUT], F16, kind="ExternalOutput")

    Exp = mybir.ActivationFunctionType.Exp

    with tile.TileContext(nc) as tc:
        with (
            tc.tile_pool(name="consts", bufs=1) as cpool,
            tc.tile_pool(name="wts", bufs=1) as wpool,
            tc.tile_pool(name="acts", bufs=1) as apool,
            tc.tile_pool(name="temps", bufs=1) as tpool,
        ):
            # ---- constants ----
            ones_t = cpool.tile([128, 512], F16, name="ones_t", tag="ones_t")
            nc.vector.memset(ones_t[:], 1.0)
            mask_t = cpool.tile([128, 128], F16, name="mask_t", tag="mask_t")
            nc.scalar.dma_start(mask_t[:], mask_d.ap())

            # ---- weights ----
            # xt/wdkv stream on the sync HWDGE ring (feed the first matmuls);
            # everything else loads in parallel on the scalar ring.
            # xt chunk 0 is split across both rings to cut first-matmul
            # latency.
            wdkv = []
            xt = []
            # Phase-1 critical stream: ONLY xt+wdkv ride the queues while the
            # latT pass runs.  Per-queue effective rate is ~150-160 GB/s
            # (in-flight credit x completion latency), so the ~316 GB/s
            # latT-pass demand is split over THREE queues (sync/scalar HWDGE
            # + gpsimd SWDGE): 224/176/176 KB per 1.79us chunk window.
            # Chunk 0 split finer so the first matmul's inputs land ~1.7us.
            for k in range(KC):
                t = wpool.tile([128, S], F16, name=f"xt{k}", tag=f"xt{k}")
                r = xt_d.ap()[128 * k:128 * (k + 1), :]
                if k == 0:
                    nc.sync.dma_start(t[:, 0:512], r[:, 0:512])
                    nc.sync.dma_start(t[:, 512:1024], r[:, 512:1024])
                    nc.scalar.dma_start(t[:, 1024:1536], r[:, 1024:1536])
                    nc.scalar.dma_start(t[:, 1536:2048], r[:, 1536:2048])
                else:
                    nc.sync.dma_start(t[:, 0:1024], r[:, 0:1024])
                    nc.scalar.dma_start(t[:, 1024:2048], r[:, 1024:2048])
                xt.append(t)
                td = wpool.tile([128, LAT], F16, name=f"wdkv{k}", tag=f"wdkv{k}")
                nc.sync.dma_start(td[:], wdkv_d.ap()[128 * k:128 * (k + 1), :])
                wdkv.append(td)
            # wuk/wuv queue on sync AFTER its xt share (arrive ~32us, needed
            # ~86us). wq/wout are deferred until after the latT pass is
            # emitted (see below) so they stream on scalar behind xt.
            wuk = []
            wuv = []
            for m in range(LAT // 128):
                t = wpool.tile([128, COLS_PER_CORE], F16, name=f"wuk{m}", tag=f"wuk{m}")
                nc.sync.dma_start(t[:], wuk_d.ap()[128 * m:128 * (m + 1), :])
                wuk.append(t)
                t = wpool.tile([128, COLS_PER_CORE], F16, name=f"wuv{m}", tag=f"wuv{m}")
                nc.sync.dma_start(t[:], wuv_d.ap()[128 * m:128 * (m + 1), :])
                wuv.append(t)
            wq = [wpool.tile([128, COLS_PER_CORE], F16, name=f"wq{k}", tag=f"wq{k}")
                  for k in range(KC)]
            wout = [wpool.tile([128, DOUT], F16, name=f"wout{h}", tag=f"wout{h}")
                    for h in range(HEADS_PER_CORE)]

            # ---- persistent activations ----
            latT = [apool.tile([128, S], F16, name=f"latT{m}", tag=f"latT{m}")
                    for m in range(LAT // 128)]
            qT = [apool.tile([128, S], F16, name=f"qT{h}", tag=f"qT{h}")
                  for h in range(HEADS_PER_CORE)]
            kT = [apool.tile([128, S], F16, name=f"kT{h}", tag=f"kT{h}")
                  for h in range(HEADS_PER_CORE)]
            # vtt[stt]: [s within tile (part), 4 heads x hd (free)]
            vtt = [apool.tile([128, 512], F16, name=f"vtt{t}", tag=f"vtt{t}")
                   for t in range(NT)]
            ctxT = [apool.tile([128, S], F16, name=f"ctxT{h}", tag=f"ctxT{h}")
                    for h in range(HEADS_PER_CORE)]

            # ================= phase 1: projections =================
            with tc.tile_pool(name="pproj", bufs=8, space="PSUM") as pproj:
                # PE warmup: HAM-warm the array while input DMAs stream in.
                # ~32 cold matmuls (~3.4us at 1.2GHz) bridges to the first
                # xt piece landing AND spans the HAM activity window.
                warm = pproj.tile([128, 512], F32, name="warm", tag="pp")
                for _ in range(110):
                    nc.tensor.matmul(warm[:, 0:128], ones_t[:, 0:128],
                                     ones_t[:, 0:128], start=True, stop=True)

                def kmajor(groups, lhs_of, rhs_of, nk, out_of, copy_eng="alt"):
                    """Accumulate len(groups) psum banks over nk chunks,
                    chunk-major so compute starts on the first DMA. The
                    drain copies alternate scalar/vector so the next pass's
                    matmuls get psum banks back at 2x the single-engine
                    copy rate."""
                    pls = [pproj.tile([128, 512], F32, name=f"pp{i}", tag="pp")
                           for i in range(len(groups))]
                    for k in range(nk):
                        for i, g in enumerate(groups):
                            nc.tensor.matmul(pls[i][:], lhs_of(k, g), rhs_of(k, g),
                                             start=(k == 0), stop=(k == nk - 1))
                    for i, g in enumerate(groups):
                        if i % 2 == 0:
                            nc.scalar.copy(out_of(g), pls[i][:])
                        else:
                            nc.vector.tensor_copy(out_of(g), pls[i][:])

                # latT = Wdkv^T xT   (8 groups: 2 m x 4 sb)
                kmajor(
                    [(m, sb) for m in range(2) for sb in range(NB)],
                    lambda k, g: wdkv[k][:, 128 * g[0]:128 * (g[0] + 1)],
                    lambda k, g: xt[k][:, 512 * g[1]:512 * (g[1] + 1)],
                    KC,
                    lambda g: latT[g[0]][:, 512 * g[1]:512 * (g[1] + 1)])

                # wq/wout stream on scalar BEHIND its xt share (FIFO per
                # ring): wq chunk k lands ~23+0.7k us, needed ~29+1.8k us.
                for k in range(KC):
                    nc.scalar.dma_start(wq[k][:],
                                        wq_d.ap()[128 * k:128 * (k + 1), :])
                for h in range(HEADS_PER_CORE):
                    nc.scalar.dma_start(wout[h][:],
                                        wout_d.ap()[128 * h:128 * (h + 1), :])

                # qT_h = Wq_h^T xT   (two batches of 8 groups: 2 h x 4 sb)
                for h0 in (0, 2):
                    kmajor(
                        [(h0 + dh, sb) for dh in range(2) for sb in range(NB)],
                        lambda k, g: wq[k][:, 128 * g[0]:128 * (g[0] + 1)],
                        lambda k, g: xt[k][:, 512 * g[1]:512 * (g[1] + 1)],
                        KC,
                        lambda g: qT[g[0]][:, 512 * g[1]:512 * (g[1] + 1)])

                # kT_h = Wuk_h^T latT
                kmajor(
                    [(h, sb) for h in range(2) for sb in range(NB)],
                    lambda k, g: wuk[k][:, 128 * g[0]:128 * (g[0] + 1)],
                    lambda k, g: latT[k][:, 512 * g[1]:512 * (g[1] + 1)],
                    2,
                    lambda g: kT[g[0]][:, 512 * g[1]:512 * (g[1] + 1)])
                kmajor(
                    [(h, sb) for h in (2, 3) for sb in range(NB)],
                    lambda k, g: wuk[k][:, 128 * g[0]:128 * (g[0] + 1)],
                    lambda k, g: latT[k][:, 512 * g[1]:512 * (g[1] + 1)],
                    2,
                    lambda g: kT[g[0]][:, 512 * g[1]:512 * (g[1] + 1)])

                # v = latT^T Wuv per S-tile, all 4 heads wide (N=512).
                # Only tiles 0-3 here (qb0 reads them); tiles 4-15 are
                # emitted as interleaved filler inside qb0's attention.
                for stt in range(4):
                    pv = pproj.tile([128, 512], F32, name="pv", tag="pp")
                    for m in range(LAT // 128):
                        nc.tensor.matmul(
                            pv[:],
                            latT[m][:, 128 * stt:128 * (stt + 1)],
                            wuv[m][:],
                            start=(m == 0), stop=(m == LAT // 128 - 1))
                    if stt % 2 == 0:
                        nc.vector.tensor_copy(vtt[stt][:], pv[:])
                    else:
                        nc.scalar.copy(vtt[stt][:], pv[:])

            # ========= phase 2: attention + interleaved out-proj =========
            # per key-tile processing on exact valid column ranges; softmax
            # denominator accumulated on DVE (exacc), one ones-matmul per
            # (h, qb) to reduce over keys + broadcast across partitions.
            # qb-outer / h-inner so each q-block's out-projection (PE-heavy,
            # ACT-idle) overlaps the next block's ACT-paced attention.
            with (
                tc.tile_pool(name="psc", bufs=4, space="PSUM") as psc,
                tc.tile_pool(name="pctx", bufs=2, space="PSUM") as pctx,
                tc.tile_pool(name="pden", bufs=2, space="PSUM") as pden,
            ):
                def emit_outproj(stt, ob, pool, tail):
                    po = pool.tile([128, 512], F32, name="po", tag="den")
                    for h in range(HEADS_PER_CORE):
                        nc.tensor.matmul(
                            po[:],
                            ctxT[h][:, 128 * stt:128 * (stt + 1)],
                            wout[h][:, 512 * ob:512 * (ob + 1)],
                            start=(h == 0), stop=(h == HEADS_PER_CORE - 1))
                    osb = tpool.tile([128, 512], F16, name="osb", tag="osb",
                                     bufs=12)
                    if tail and (stt + ob) % 2 == 0:
                        nc.scalar.copy(osb[:], po[:])
                        st_eng = nc.scalar
                    else:
                        nc.vector.tensor_copy(osb[:], po[:])
                        st_eng = nc.sync
                    st_eng.dma_start(
                        out_d.ap()[128 * stt:128 * (stt + 1),
                                   512 * ob:512 * (ob + 1)],
                        osb[:])

                # out-projection of block qb-1 is interleaved into block
                # qb's attention (one (stt, ob) group every few key-tiles)
                # so the PE fills exp-latency bubbles with out-proj matmuls
                # and (h, qb) boundaries never drain the pipeline.
                def emit_vop(stt):
                    pv = psc.tile([128, 512], F32, name="pv", tag="sc")
                    for m in range(LAT // 128):
                        nc.tensor.matmul(
                            pv[:],
                            latT[m][:, 128 * stt:128 * (stt + 1)],
                            wuv[m][:],
                            start=(m == 0), stop=(m == LAT // 128 - 1))
                    if stt % 2 == 0:
                        nc.vector.tensor_copy(vtt[stt][:], pv[:])
                    else:
                        nc.scalar.copy(vtt[stt][:], pv[:])

                for qb in range(NB):
                    if qb > 0:
                        ops = [(lambda s=stt, o=ob:
                                emit_outproj(s, o, pden, tail=False))
                               for stt in range(4 * (qb - 1), 4 * qb)
                               for ob in range(NB)]
                    else:
                        # fill qb0's exp-latency bubbles with the remaining
                        # v-tile projections (first needed by qb1)
                        ops = [(lambda s=stt: emit_vop(s))
                               for stt in range(4, NT)]
                    nkt = 4 * qb + 4
                    total_kts = HEADS_PER_CORE * nkt
                    emitted = 0
                    done_kts = 0
                    for h in range(HEADS_PER_CORE):
                        ps_ctx = pctx.tile([128, 512], F32, name="ps_ctx", tag="ctx")
                        exacc = tpool.tile([128, 512], F16, name="exacc",
                                           tag="exacc", bufs=3)
                        for kt in range(nkt):
                            dj = kt - 4 * qb
                            c = 128 * dj if dj > 0 else 0
                            ps_sc = psc.tile([128, 512], F32, name="ps_sc",
                                             tag="sc")
                            ex = tpool.tile([128, 512], F16, name="ex", tag="ex",
                                            bufs=6)
                            nc.tensor.matmul(
                                ps_sc[:, c:512],
                                kT[h][:, 128 * kt:128 * (kt + 1)],
                                qT[h][:, 512 * qb + c:512 * (qb + 1)],
                                start=True, stop=True)
                            nc.scalar.activation(ex[:, c:512], ps_sc[:, c:512],
                                                 Exp, scale=SCALE)
                            if dj >= 0:
                                nc.vector.tensor_mul(
                                    ex[:, c:c + 128], ex[:, c:c + 128], mask_t[:])
                            if kt == 0:
                                nc.vector.tensor_copy(exacc[:], ex[:])
                            else:
                                nc.vector.tensor_add(exacc[:, c:512],
                                                     exacc[:, c:512],
                                                     ex[:, c:512])
                            nc.tensor.matmul(
                                ps_ctx[:, c:512],
                                vtt[kt][:, 128 * h:128 * (h + 1)],
                                ex[:, c:512],
                                start=(kt == 0), stop=(kt == nkt - 1),
                                skip_group_check=True)
                            done_kts += 1
                            while ops and emitted < (done_kts * len(ops)) // total_kts:
                                ops[emitted]()
                                emitted += 1
                        ps_den = pden.tile([128, 512], F32, name="ps_den",
                                           tag="den")
                        nc.tensor.matmul(ps_den[:], ones_t[:, 0:128], exacc[:],
                                         start=True, stop=True)
                        rden = tpool.tile([128, 512], F32, name="rden", tag="rden",
                                          bufs=2)
                        nc.vector.reciprocal_approx_fast(rden[:], ps_den[:])
                        nc.vector.tensor_mul(ctxT[h][:, 512 * qb:512 * (qb + 1)],
                                             ps_ctx[:], rden[:])
                    while ops and emitted < len(ops):
                        ops[emitted]()
                        emitted += 1

                # final q-block's out-projection (nothing left to overlap)
                for stt in range(4 * (NB - 1), 4 * NB):
                    for ob in range(NB):
                        emit_outproj(stt, ob, pden, tail=True)

    nc.compile()
    return nc


def _get_nc():
    if "nc" not in _CACHE:
        _CACHE["nc"] = _build()
    return _CACHE["nc"]


def _make_in_maps(x, W_query, W_DKV, W_UK, W_UV, W_out):
    mask = np.triu(np.ones((128, 128), dtype=np.float16))
    wdkv16 = W_DKV.astype(np.float16)
    xT16 = [x[b].T.astype(np.float16) for b in range(B)]
    in_maps = []
    for c in range(NCORES):
        b = c // 4
        g = c % 4
        cols = slice(512 * g, 512 * (g + 1))
        in_maps.append({
            "xt": xT16[b],
            "wq": W_query[:, cols].astype(np.float16),
            "wdkv": wdkv16,
            "wuk": W_UK[:, cols].astype(np.float16),
            "wuv": W_UV[:, cols].astype(np.float16),
            "wout": W_out[cols, :].astype(np.float16),
            "mask": mask,
        })
    return in_maps


def run_on_device(x, W_query, W_DKV, W_UK, W_UV, W_out, **run_kwargs):
    nc = _get_nc()
    in_maps = _make_in_maps(x, W_query, W_DKV, W_UK, W_UV, W_out)
    return bass_utils.run_bass_kernel_spmd(
        nc, in_maps, core_ids=list(range(NCORES)), **run_kwargs)


def kernel(x, W_query, W_DKV, W_UK, W_UV, W_out, b_out):
    x = np.asarray(x, dtype=np.float32)
    W_query = np.asarray(W_query, dtype=np.float32)
    W_DKV = np.asarray(W_DKV, dtype=np.float32)
    W_UK = np.asarray(W_UK, dtype=np.float32)
    W_UV = np.asarray(W_UV, dtype=np.float32)
    W_out = np.asarray(W_out, dtype=np.float32)
    b_out = np.asarray(b_out, dtype=np.float32)

    res = None
    for attempt in range(3):
        try:
            res = run_on_device(x, W_query, W_DKV, W_UK, W_UV, W_out)
            break
        except Exception:
            if attempt == 2:
                raise
    out = np.empty((B, S, DOUT), dtype=np.float32)
    for b in range(B):
        acc = res.results[4 * b]["out"].astype(np.float32)
        for g in range(1, 4):
            acc += res.results[4 * b + g]["out"].astype(np.float32)
        out[b] = acc + b_out[None, :]
    return out

